# revision 1
# baseline (speedup 1.0000x reference)
"""Trainium2 Bass kernel for a ResNet BasicBlock (dense CNN, sync-BN).

Reference computation (training-mode BN, batch stats over (N,H,W)):
    h = conv3x3(x, W1) * mask1            # structured channel pruning
    h = relu(bn(h, gamma1, beta1))
    h = conv3x3(h, W2) * mask2
    h = bn(h, gamma2, beta2)
    out = relu(h + x)                      # identity shortcut

Shapes: x [32, 256, 56, 56] f32, W [256, 256, 3, 3] f32.

Strategy: data-parallel over batch N across 8 NeuronCores (4 images per
core), weights replicated.  BN batch statistics are synchronized with a
tiny (2 KB) AllReduce of per-channel (sum, sum-of-squares) pairs.

Per-core layout:
  - Channels are split into two 128-partition halves (C=256 = 2*128).
  - Conv inputs live in SBUF as zero-padded 58x58 bf16 planes (row
    stride 58), so each of the 9 taps of the 3x3 conv is a plain offset
    shift: one matmul per (tap, ci-half) accumulating into PSUM.
  - Each image's 56 output rows are produced in 7 chunks of 8 rows
    (464 = 8*58 contiguous padded positions) so a chunk fits a PSUM
    bank; the interior (56 of 58 columns) is copied out with a strided
    AP, discarding the junk computed at the pad columns.
  - Masks are folded into the weights on the host (zero rows), so
    masked output channels are exactly zero everywhere.
"""

import numpy as np
import ml_dtypes

# ---- problem constants (hardcoded; kernel.py must be self-contained) ----
N_TOT, C, H, W = 32, 256, 56, 56
N_CORES = 8
NL = N_TOT // N_CORES          # images per core
PW = H + 2                     # padded row stride (58)
PLANE = PW * PW + 4            # padded plane floats + 4 spare for tap overreads
STRIP0 = PW + 1                # first interior output position (59)
CHUNK = 8 * PW                 # 464: 8 output rows per chunk
NCHUNK = 7                     # 7 chunks * 8 rows = 56 rows
HW = H * W                     # 3136
HALF_ROWS = 28                 # row granularity for x/out streaming DMAs
HALF_ELEMS = HALF_ROWS * W     # 1568
COUNT = N_TOT * HW             # sync-BN element count per channel
EPS = 1e-5

_BF16 = ml_dtypes.bfloat16

_cache = {}


def _pack_weights(W1, W2, mask1, mask2):
    """-> [128, 72*128] bf16: [i, (conv,ky,kx,ci,co), o] with masks folded."""
    Wm = np.stack([W1 * mask1[:, None, None, None],
                   W2 * mask2[:, None, None, None]]).astype(np.float32)
    # [conv, O, I, 3, 3] -> [conv, co, o, ci, i, ky, kx]
    Wr = Wm.reshape(2, 2, 128, 2, 128, 3, 3)
    # -> [conv, ky, kx, ci, co, i, o]
    A = Wr.transpose(0, 5, 6, 3, 1, 4, 2)
    # -> [i, t, o] -> [128, 72*128]
    B = np.ascontiguousarray(A.transpose(5, 0, 1, 2, 3, 4, 6)).reshape(128, 72 * 128)
    return B.astype(_BF16)


def _t_index(conv, ky, kx, ci, co):
    return co + 2 * (ci + 2 * (kx + 3 * (ky + 3 * conv)))


def _pack_aff(gamma1, beta1, gamma2, beta2):
    cols = [gamma1, beta1, gamma2, beta2]
    out = np.empty((128, 8), np.float32)
    for k, v in enumerate(cols):
        v = np.asarray(v, np.float32).reshape(2, 128)
        out[:, 2 * k] = v[0]
        out[:, 2 * k + 1] = v[1]
    return out


def _build():
    import concourse.bass as bass_mod
    import concourse.bacc as bacc
    import concourse.mybir as mybir
    import concourse.tile as tile

    f32 = mybir.dt.float32
    bf16 = mybir.dt.bfloat16
    AX = mybir.AxisListType
    ALU = mybir.AluOpType
    AF = mybir.ActivationFunctionType

    nc = bacc.Bacc("TRN2", target_bir_lowering=False, debug=False,
                   num_devices=N_CORES)

    x_d = nc.dram_tensor("x", [NL, C, H, W], f32, kind="ExternalInput")
    wt_d = nc.dram_tensor("wt", [128, 72 * 128], bf16, kind="ExternalInput")
    aff_d = nc.dram_tensor("aff", [128, 8], f32, kind="ExternalInput")
    out_d = nc.dram_tensor("out", [NL, C, H, W], f32, kind="ExternalOutput")

    groups = [list(range(N_CORES))]

    def interior(tile_ap, base, nrows):
        """[128, nrows, 56] strided view (row stride PW) starting at `base`."""
        v = tile_ap[:, base:base + nrows * PW].rearrange(
            "p (r c) -> p r c", c=PW)
        return v[:, :, 0:W]

    with tile.TileContext(nc) as tc:
        import contextlib
        with contextlib.ExitStack() as ctx:
            const = ctx.enter_context(tc.tile_pool(name="const", bufs=1))
            psum = ctx.enter_context(tc.tile_pool(name="psum", bufs=6, space="PSUM"))
            xst = ctx.enter_context(tc.tile_pool(name="xst", bufs=2))
            otp = ctx.enter_context(tc.tile_pool(name="otp", bufs=2))
            sqp = ctx.enter_context(tc.tile_pool(name="sqp", bufs=2))

            wt_sb = const.tile([128, 72 * 128], bf16, tag="wt", name="wt")
            nc.sync.dma_start(wt_sb[:], wt_d[:])
            aff_sb = const.tile([128, 8], f32, tag="aff", name="aff")
            nc.sync.dma_start(aff_sb[:], aff_d[:])

            # ---- cross-core stats exchange plumbing (SBUF remote DMA) ----
            # Each of the 4 BN-stat exchanges broadcasts this core's [128,2]
            # (sum, sumsq) to all 7 peers with XOR-relative dests; slot d of
            # the receive tile gets the copy from core (me ^ d).  Hardware
            # remote sems count arrivals (2 per transfer -> wait >= 14).
            rsem = [nc.alloc_semaphore(f"rst{i}") for i in range(4)]
            lsem = nc.alloc_semaphore("lst")
            _gp_prev = [None]
            # waits that the single-core scheduling simulator cannot satisfy
            # (remote increments); attached to the instructions after the
            # TileContext exits, before nc.compile()
            deferred_waits = []

            def gp_order(bi):
                if _gp_prev[0] is not None:
                    bass_mod._add_dep_helper(bi.ins, _gp_prev[0].ins,
                                             sync=False,
                                             reason="stats-exchange order")
                _gp_prev[0] = bi
                return bi

            # register the entry barrier (prelude AllGather increments
            # _bir_kernel_barrier_sem once every core has entered the NEFF)
            nc._bir_kernel_barrier_sem_replica_groups.extend(
                set(g) for g in groups)

            def defer_wait(bi, sem, val):
                # reserve the wait slot with an always-satisfied threshold so
                # the scheduling simulator passes; patched to `val` later
                bi._wait_ge(sem, 0)
                deferred_waits.append((bi, sem, val))
                return bi

            # sems persist across NEFF executions: clear them as soon as all
            # cores have entered (peers send >100us later, after conv1)
            for i, s in enumerate(rsem + [lsem]):
                cl = gp_order(nc.gpsimd.sem_clear(s))
                if i == 0:
                    defer_wait(cl, nc._bir_kernel_barrier_sem,
                               nc.bir_kernel_barrier_sem_inc)

            # persistent per-image planes
            x_pad = [[const.tile([128, PLANE], bf16, tag=f"xp{j}_{n}", name=f"xp{j}_{n}")
                      for n in range(NL)] for j in range(2)]
            h1_pad = [[const.tile([128, PLANE], bf16, tag=f"h1{j}_{n}", name=f"h1{j}_{n}")
                       for n in range(NL)] for j in range(2)]
            h2 = [[const.tile([128, HW], bf16, tag=f"h2{j}_{n}", name=f"h2{j}_{n}")
                   for n in range(NL)] for j in range(2)]

            # zero the non-interior positions of every padded plane:
            #  (a) [0, 59)  (b) pad-column pairs  (c) [3307, PLANE)
            for planes in (x_pad, h1_pad):
                for j in range(2):
                    for n in range(NL):
                        t = planes[j][n]
                        nc.vector.memset(t[:, 0:STRIP0], 0.0)
                        pairs = t[:, 2 * PW - 1:2 * PW - 1 + 56 * PW].rearrange(
                            "p (r c) -> p r c", c=PW)[:, :, 0:2]
                        nc.vector.memset(pairs, 0.0)
                        nc.vector.memset(t[:, STRIP0 + 56 * PW:PLANE], 0.0)

            # stats accumulators: one column per (image, chunk)
            acc = {(b, s, j): const.tile([128, NL * NCHUNK], f32,
                                         tag=f"acc{b}{s}{j}", name=f"acc{b}{s}{j}")
                   for b in (1, 2) for s in ("s", "q") for j in range(2)}

            # ---- head: stream x in, cast-scatter into padded bf16 planes ----
            for n in range(NL):
                for j in range(2):
                    for rh in range(2):
                        r0 = rh * HALF_ROWS
                        xs = xst.tile([128, HALF_ELEMS], f32, tag="xs", name="xs")
                        nc.sync.dma_start(
                            xs[:], x_d[n, j * 128:(j + 1) * 128, r0:r0 + HALF_ROWS, :])
                        dst = interior(x_pad[j][n], (r0 + 1) * PW + 1, HALF_ROWS)
                        src = xs[:, :].rearrange("p (r c) -> p r c", c=W)
                        nc.vector.tensor_copy(dst, src)

            # ---- conv + stats emission helper (one output half) ----
            def conv(conv_i, src_planes, dst_j_n_base, bn_i, j):
                """src_planes[ci][n]: padded bf16 planes; writes dst via
                dst_j_n_base(j, n, k) -> (tile, interior_view) and stats."""
                for n in range(NL):
                    if True:
                        for k in range(NCHUNK):
                            pt = psum.tile([128, 8 * W], f32, tag="ps", name="ps")
                            idx = 0
                            for ci in range(2):
                                for ky in range(3):
                                    for kx in range(3):
                                        t = _t_index(conv_i, ky, kx, ci, j)
                                        dq = (ky - 1) * PW + (kx - 1)
                                        off = STRIP0 + CHUNK * k + dq
                                        rhs = src_planes[ci][n][
                                            :, off:off + CHUNK].rearrange(
                                            "p (r c) -> p r c", c=PW)[:, :, 0:W]
                                        nc.tensor.matmul(
                                            pt[:],
                                            wt_sb[:, t * 128:(t + 1) * 128],
                                            rhs,
                                            start=(idx == 0), stop=(idx == 17))
                                        idx += 1
                            src_int = pt[:, 0:8 * W].rearrange(
                                "p (r c) -> p r c", c=W)
                            dst_int = dst_j_n_base(j, n, k)
                            col = n * NCHUNK + k
                            # copy + sum on ACT; square + sumsq also on ACT so
                            # the DVE stream stays free for AR-gated work
                            nc.scalar.activation(
                                dst_int, src_int, AF.Copy,
                                accum_out=acc[(bn_i, "s", j)][:, col:col + 1])
                            sq = sqp.tile([128, 8 * W], f32, tag="sq", name="sq")
                            last = nc.scalar.activation(
                                sq[:, :].rearrange("p (r c) -> p r c", c=W),
                                dst_int, AF.Square,
                                accum_out=acc[(bn_i, "q", j)][:, col:col + 1])
                return last

            # ---- BN stats: local reduce + send to all peers (one half).
            # Emitted right after the conv that produced the stats so the
            # transfer latency overlaps the other half's conv compute. ----
            def bn_stats_send(bn_i, j, ex):
                sfx = f"{bn_i}{j}"
                packed = const.tile([128, 2], f32, tag=f"pk{sfx}", name=f"pk{sfx}")
                nc.vector.tensor_reduce(
                    packed[:, 0:1], acc[(bn_i, "s", j)][:], axis=AX.X, op=ALU.add)
                nc.vector.tensor_reduce(
                    packed[:, 1:2], acc[(bn_i, "q", j)][:], axis=AX.X, op=ALU.add)
                rv = const.tile([128, 16], f32, tag=f"rv{ex}", name=f"rv{ex}")
                nc.vector.tensor_copy(rv[:, 0:2], packed[:])   # own slot (d=0)
                for d in range(1, 8):
                    rd = [None] * 8
                    rd[d] = (0, d)
                    gp_order(nc.gpsimd.remote_dma_broadcast(
                        rv[:, 2 * d:2 * d + 2], packed[:],
                        remote_sem=rsem[ex], local_sem=lsem, rdests=rd))
                gp_order(nc.gpsimd.trigger_dma(count=None))
                return rv

            # ---- wait for all 8 contributions, sum slots -> global stats.
            # MUST be emitted after every conv-phase op of DVE's stream: any
            # long-blocked op sitting ahead of conv work in an engine's
            # in-order stream stalls cross-engine waits that point past it.
            def bn_stats_recv(bn_i, j, ex, rv, after):
                sfx = f"{bn_i}{j}"
                gl = const.tile([128, 2], f32, tag=f"gl{sfx}", name=f"gl{sfx}")
                red = nc.vector.tensor_reduce(
                    gl[:], rv[:, 0:16].rearrange("p (s c) -> p c s", c=2),
                    axis=AX.X, op=ALU.add)
                defer_wait(red, rsem[ex], 14)
                # pin the whole arrival-gated chain behind the conv phase so
                # the scheduler cannot interleave it into engine streams that
                # conv-era waits threshold against
                bass_mod._add_dep_helper(red.ins, after.ins, sync=True,
                                         reason="recv after conv phase")
                return gl

            # ---- BN affine from global stats — DVE-only (no ACT ops, so
            # waiting on the collective never blocks ACT's copy stream).
            # rsqrt via the fast-inverse-sqrt bit trick + 2 Newton steps. ----
            def bn_affine_finish(bn_i, j, gl, g_col, b_col):
                sfx = f"{bn_i}{j}"
                mean = const.tile([128, 1], f32, tag=f"mean{sfx}", name=f"mean{sfx}")
                nc.vector.tensor_scalar_mul(mean[:], gl[:, 0:1], 1.0 / COUNT)
                var = const.tile([128, 1], f32, tag=f"var{sfx}", name=f"var{sfx}")
                nc.vector.tensor_tensor(var[:], mean[:], mean[:], ALU.mult)
                nc.vector.scalar_tensor_tensor(
                    var[:], gl[:, 1:2], 1.0 / COUNT, var[:],
                    ALU.mult, ALU.subtract)
                nc.vector.tensor_scalar_add(var[:], var[:], EPS)
                y = const.tile([128, 1], f32, tag=f"y{sfx}", name=f"y{sfx}")
                vh = const.tile([128, 1], f32, tag=f"vh{sfx}", name=f"vh{sfx}")
                tmp = const.tile([128, 1], f32, tag=f"tm{sfx}", name=f"tm{sfx}")
                iv = var[:].bitcast(mybir.dt.int32)
                yi = y[:].bitcast(mybir.dt.int32)
                nc.vector.tensor_scalar(yi, iv, 1, None, ALU.arith_shift_right)
                nc.vector.tensor_scalar(yi, yi, -1, None, ALU.bitwise_xor)
                nc.vector.tensor_scalar(yi, yi, 0x5f3759df + 1, None, ALU.add)
                nc.vector.tensor_scalar_mul(vh[:], var[:], 0.5)
                for _ in range(2):
                    nc.vector.tensor_tensor(tmp[:], y[:], y[:], ALU.mult)
                    nc.vector.tensor_tensor(tmp[:], tmp[:], vh[:], ALU.mult)
                    nc.vector.tensor_scalar(tmp[:], tmp[:], -1.0, 1.5,
                                            ALU.mult, ALU.add)
                    nc.vector.tensor_tensor(y[:], y[:], tmp[:], ALU.mult)
                sc = const.tile([128, 1], f32, tag=f"sc{sfx}", name=f"sc{sfx}")
                nc.vector.tensor_tensor(sc[:], aff_sb[:, g_col + j:g_col + j + 1],
                                        y[:], ALU.mult)
                bi = const.tile([128, 1], f32, tag=f"bi{sfx}", name=f"bi{sfx}")
                nc.vector.tensor_tensor(bi[:], mean[:], sc[:], ALU.mult)
                nc.vector.tensor_tensor(bi[:], aff_sb[:, b_col + j:b_col + j + 1],
                                        bi[:], ALU.subtract)
                return sc, bi

            def h1_dst(j, n, k):
                return interior(h1_pad[j][n], (1 + 8 * k) * PW + 1, 8)

            def h2_dst(j, n, k):
                return h2[j][n][:, 8 * k * W:(8 * k + 8) * W].rearrange(
                    "p (r c) -> p r c", c=W)

            def tail(j, s2, b2, use_act):
                # out = relu(s2*h2 + b2 + x); x from resident bf16 planes.
                # use_act=False → all-DVE (so waiting on AR2 never blocks
                # ACT's copy stream while the other half is still convolving).
                # use_act=True (last phase, ACT idle) → relu on ACT, pipelined.
                for m, (n, rh) in enumerate((n, rh) for n in range(NL)
                                            for rh in range(2)):
                    r0 = rh * HALF_ROWS
                    xv = interior(x_pad[j][n], (r0 + 1) * PW + 1, HALF_ROWS)
                    h2v = h2[j][n][:, r0 * W:r0 * W + HALF_ELEMS].rearrange(
                        "p (r c) -> p r c", c=W)
                    pool = otp if m % 2 == 0 else xst
                    ot = pool.tile([128, HALF_ELEMS], f32,
                                   tag="ot" if m % 2 == 0 else "xs", name="ot")
                    otv = ot[:, :].rearrange("p (r c) -> p r c", c=W)
                    nc.vector.scalar_tensor_tensor(
                        otv, h2v, s2[:], xv, ALU.mult, ALU.add)
                    if use_act:
                        nc.scalar.activation(ot[:], ot[:], AF.Relu,
                                             bias=b2[:], scale=1.0)
                    else:
                        nc.vector.tensor_scalar(ot[:], ot[:], b2[:], 0.0,
                                                ALU.add, ALU.max)
                    nc.sync.dma_start(
                        out_d[n, j * 128:(j + 1) * 128, r0:r0 + HALF_ROWS, :],
                        ot[:])

            # ---- phase schedule: j-outer; stats are SENT right after the
            # conv half that produced them (transfer overlaps the other
            # half's conv), but RECEIVED only after all conv emission so no
            # arrival-gated op sits ahead of conv work in an engine stream.
            conv(0, x_pad, h1_dst, 1, 0)           # conv1 half 0
            rv1_0 = bn_stats_send(1, 0, 0)
            c1_last = conv(0, x_pad, h1_dst, 1, 1)  # conv1 half 1
            rv1_1 = bn_stats_send(1, 1, 1)
            gl1_0 = bn_stats_recv(1, 0, 0, rv1_0, c1_last)
            s1_0, b1_0 = bn_affine_finish(1, 0, gl1_0, 0, 2)
            gl1_1 = bn_stats_recv(1, 1, 1, rv1_1, c1_last)  # peer-skew wait
            s1_1, b1_1 = bn_affine_finish(1, 1, gl1_1, 0, 2)
            # BN1 apply + relu in place (ACT) — n-major so conv2 image 0
            # unblocks first
            for n in range(NL):
                for j, (s1, b1) in ((0, (s1_0, b1_0)), (1, (s1_1, b1_1))):
                    v = interior(h1_pad[j][n], STRIP0, H)
                    nc.scalar.activation(v, v, AF.Relu,
                                         bias=b1[:], scale=s1[:])
            conv(1, h1_pad, h2_dst, 2, 0)          # conv2 half 0
            rv2_0 = bn_stats_send(2, 0, 2)
            c2_last = conv(1, h1_pad, h2_dst, 2, 1)  # conv2 half 1
            rv2_1 = bn_stats_send(2, 1, 3)
            gl2_0 = bn_stats_recv(2, 0, 2, rv2_0, c2_last)
            s2_0, b2_0 = bn_affine_finish(2, 0, gl2_0, 4, 6)
            tail(0, s2_0, b2_0, use_act=True)      # overlaps half-1 peer skew
            gl2_1 = bn_stats_recv(2, 1, 3, rv2_1, c2_last)
            s2_1, b2_1 = bn_affine_finish(2, 1, gl2_1, 4, 6)
            tail(1, s2_1, b2_1, use_act=True)

    # patch the reserved wait slots to their real thresholds now that
    # scheduling is done (the single-core scheduling simulator cannot
    # satisfy remote increments)
    for bi, sem, val in deferred_waits:
        patched = False
        for w in bi.ins.sync_info.on_wait:
            if w.id == sem.num and w.wait_value == 0:
                w.wait_value = val
                patched = True
                break
        assert patched, f"deferred wait not found on {bi.ins.name}"

    nc.compile()
    return nc


def kernel(x, W1, W2, gamma1, beta1, gamma2, beta2, mask1, mask2,
           _trace=False, _trace_kwargs=None):
    from concourse.bass_utils import run_bass_kernel_spmd

    if "nc" not in _cache:
        _cache["nc"] = _build()
    nc = _cache["nc"]

    wt = _pack_weights(np.asarray(W1, np.float32), np.asarray(W2, np.float32),
                       np.asarray(mask1, np.float32), np.asarray(mask2, np.float32))
    aff = _pack_aff(gamma1, beta1, gamma2, beta2)
    x = np.ascontiguousarray(np.asarray(x, np.float32))

    in_maps = [{"x": x[i * NL:(i + 1) * NL], "wt": wt, "aff": aff}
               for i in range(N_CORES)]
    kw = {}
    if _trace:
        kw = dict(trace=True, **(_trace_kwargs or {}))
    res = run_bass_kernel_spmd(nc, in_maps, core_ids=list(range(N_CORES)), **kw)
    out = np.concatenate([res.results[i]["out"] for i in range(N_CORES)], axis=0)
    _cache["last_results"] = res
    return out



# revision 2
# speedup vs baseline: 1.0240x; 1.0240x over previous
"""Trainium2 Bass kernel for a ResNet BasicBlock (dense CNN, sync-BN).

Reference computation (training-mode BN, batch stats over (N,H,W)):
    h = conv3x3(x, W1) * mask1            # structured channel pruning
    h = relu(bn(h, gamma1, beta1))
    h = conv3x3(h, W2) * mask2
    h = bn(h, gamma2, beta2)
    out = relu(h + x)                      # identity shortcut

v2 exploits the mask sparsity: only K1=|mask1| conv1 outputs and
K2=|mask2| conv2 outputs are live (masked channels are exactly zero
through the block since beta=0, and masked conv2 outputs reduce to
relu(x + beta2)).

  - conv1: 128-channel main group (direct conv, 18 mm/chunk) + overflow
    group (K1-128 channels) M-packed: 2 matmuls per contiguous 464-col
    chunk produce per-(co,tap) partials for all 9 taps at once, then 9
    identity matmuls per interior chunk recombine them with tap-shifted
    reads (zero-padding falls out of the zero-padded planes).
  - conv2 contracts over only K1 inputs: 9 matmuls over the 128 main
    channels plus ONE K=9*(K1-128) matmul over pre-shifted replicas of
    the overflow planes, for both the 128-wide main output group and the
    (K2-128)-wide overflow group: 20 mm/chunk vs 36 unpruned.
  - channels host-permuted so conv2-kept channels sit in contiguous
    partition blocks; out is written permuted, un-permuted on the host.
    Masked outputs (relu(x+beta2)) have no BN2 dep: computed mid-kernel.

Sync-BN: 4 tiny XOR-relative remote-DMA all-gathers (one per channel
group), each sent right after its group's convs so the transfer hides
under remaining conv work; the 7 broadcasts per exchange spread over 4
SWDGE queues to shorten the descriptor burst.
"""

import numpy as np
import ml_dtypes

N_TOT, C, H, W = 32, 256, 56, 56
N_CORES = 8
NL = N_TOT // N_CORES
PW = H + 2                     # 58
PLANE = PW * PW + 4            # 3368
GUARD = 64
STRIP0 = PW + 1                # 59
CHUNK = 8 * PW                 # 464
NCHUNK = 7
NBCH = 8                       # ceil(PLANE/CHUNK): contiguous chunks
HW = H * W                     # 3136
HALF_ROWS = 28
HALF_ELEMS = HALF_ROWS * W
QUAD_ROWS = 14
QUAD_ELEMS = QUAD_ROWS * W
COUNT = N_TOT * HW
EPS = 1e-5

_BF16 = ml_dtypes.bfloat16
_cache = {}


def _plan(mask1, mask2, beta1):
    m1 = np.asarray(mask1) != 0
    live1 = m1 | (np.maximum(np.asarray(beta1), 0.0) > 0)
    kept1 = np.where(live1)[0]
    kept2 = np.where(np.asarray(mask2) != 0)[0]
    masked2 = np.where(np.asarray(mask2) == 0)[0]
    assert 128 < len(kept1) <= 128 + 14, f"K1={len(kept1)} unsupported"
    assert 128 < len(kept2) <= 128 + 14, f"K2={len(kept2)} unsupported"
    k1m, k1o = kept1[:128], kept1[128:]
    k2m, k2o = kept2[:128], kept2[128:]
    P = np.concatenate([k2m, k2o, masked2])
    assert len(P) == C
    return k1m, k1o, k2m, k2o, masked2, P


def _pack(W1, W2, gamma1, beta1, gamma2, beta2, mask1, mask2):
    k1m, k1o, k2m, k2o, masked2, P = _plan(mask1, mask2, beta1)
    B1, B2 = len(k1o), len(k2o)
    W1 = np.asarray(W1, np.float32) * (np.asarray(mask1, np.float32) != 0)[:, None, None, None]
    W2 = np.asarray(W2, np.float32) * (np.asarray(mask2, np.float32) != 0)[:, None, None, None]

    cols, offs = [], {}

    def add(name, block):
        r, c = block.shape
        b = np.zeros((128, c), np.float32)
        b[:r] = block
        offs[name] = sum(x.shape[1] for x in cols)
        cols.append(b)

    for j in range(2):
        for t in range(9):
            ty, tx = t // 3, t % 3
            add(f"c1m_{j}_{t}",
                W1[np.ix_(k1m, P[j * 128:(j + 1) * 128])][:, :, ty, tx].T)
    for j in range(2):
        blk = np.zeros((128, 9 * B1), np.float32)
        for t in range(9):
            ty, tx = t // 3, t % 3
            for c in range(B1):
                blk[:, 9 * t + c] = W1[k1o[c], P[j * 128:(j + 1) * 128], ty, tx]
        add(f"c1o_{j}", blk)
    # tap-selector blocks for the recomb matmuls: sel_t [9*B1, B1] picks
    # rows (c,t)=9t+c of the packed partials (rhs must start at partition 0)
    for t in range(9):
        blk = np.zeros((9 * B1, B1), np.float32)
        for c in range(B1):
            blk[9 * t + c, c] = 1.0
        add(f"sel_{t}", blk)
    for t in range(9):
        ty, tx = t // 3, t % 3
        add(f"c2m_{t}", W2[np.ix_(k2m, k1m)][:, :, ty, tx].T)
    blk = np.zeros((9 * B1, 128), np.float32)
    for t in range(9):
        ty, tx = t // 3, t % 3
        for c in range(B1):
            blk[9 * t + c, :] = W2[k2m, k1o[c], ty, tx]
    add("c2mo", blk)
    for t in range(9):
        ty, tx = t // 3, t % 3
        add(f"c2b_{t}", W2[np.ix_(k2o, k1m)][:, :, ty, tx].T)
    blk = np.zeros((9 * B1, B2), np.float32)
    for t in range(9):
        ty, tx = t // 3, t % 3
        for c in range(B1):
            blk[9 * t + c, :] = W2[k2o, k1o[c], ty, tx]
    add("c2bo", blk)

    wt = np.concatenate(cols, axis=1).astype(_BF16)

    aff = np.zeros((128, 16), np.float32)
    g1, b1 = np.asarray(gamma1, np.float32), np.asarray(beta1, np.float32)
    g2, b2 = np.asarray(gamma2, np.float32), np.asarray(beta2, np.float32)
    aff[:, 0], aff[:, 1] = g1[k1m], b1[k1m]
    aff[:B1, 2], aff[:B1, 3] = g1[k1o], b1[k1o]
    aff[:, 4], aff[:, 5] = g2[k2m], b2[k2m]
    aff[:B2, 6], aff[:B2, 7] = g2[k2o], b2[k2o]
    aff[:B2, 8] = b2[k2o]              # rows 0:B2 computed but never output
    aff[B2:, 8] = b2[masked2]          # aligned with x half-1 rows B2:128
    return wt, aff, (k1m, k1o, k2m, k2o, masked2, P), offs


def _build(B1, B2, offs, ncol):
    import concourse.bass as bass_mod
    import concourse.bacc as bacc
    import concourse.mybir as mybir
    import concourse.tile as tile

    f32 = mybir.dt.float32
    bf16 = mybir.dt.bfloat16
    AX = mybir.AxisListType
    ALU = mybir.AluOpType
    AF = mybir.ActivationFunctionType

    NB1 = 9 * B1
    nc = bacc.Bacc("TRN2", target_bir_lowering=False, debug=False,
                   num_devices=N_CORES, num_swdge_queues=4)

    x_d = nc.dram_tensor("x", [NL, C, H, W], f32, kind="ExternalInput")
    wt_d = nc.dram_tensor("wt", [128, ncol], bf16, kind="ExternalInput")
    aff_d = nc.dram_tensor("aff", [128, 16], f32, kind="ExternalInput")
    out_d = nc.dram_tensor("out", [NL, C, H, W], f32, kind="ExternalOutput")

    groups = [list(range(N_CORES))]

    def interior(t, base, nrows):
        v = t[:, base:base + nrows * PW].rearrange("p (r c) -> p r c", c=PW)
        return v[:, :, 0:W]

    with tile.TileContext(nc) as tc:
        import contextlib
        with contextlib.ExitStack() as ctx:
            const = ctx.enter_context(tc.tile_pool(name="const", bufs=1))
            psA = ctx.enter_context(tc.tile_pool(name="psA", bufs=5, space="PSUM"))
            psB = ctx.enter_context(tc.tile_pool(name="psB", bufs=1, space="PSUM"))
            psR = ctx.enter_context(tc.tile_pool(name="psR", bufs=2, space="PSUM"))
            otp = ctx.enter_context(tc.tile_pool(name="otp", bufs=2))
            sqp = ctx.enter_context(tc.tile_pool(name="sqp", bufs=1))
            sbp = ctx.enter_context(tc.tile_pool(name="sbp", bufs=1))
            prp = ctx.enter_context(tc.tile_pool(name="prp", bufs=2))

            wt_sb = const.tile([128, ncol], bf16, tag="wt", name="wt")
            nc.sync.dma_start(wt_sb[:], wt_d[:, :])
            aff_sb = const.tile([128, 16], f32, tag="aff", name="aff")
            nc.sync.dma_start(aff_sb[:], aff_d[:])

            def wcol(name, r, cw):
                o = offs[name]
                return wt_sb[0:r, o:o + cw]

            # ---- cross-core stats exchange plumbing ----
            rsem = [nc.alloc_semaphore(f"rst{i}") for i in range(4)]
            lsem = nc.alloc_semaphore("lst")
            _gp_prev = [None]
            deferred_waits = []

            def gp_order(bi):
                if _gp_prev[0] is not None:
                    bass_mod._add_dep_helper(bi.ins, _gp_prev[0].ins,
                                             sync=False, reason="stats order")
                _gp_prev[0] = bi
                return bi

            nc._bir_kernel_barrier_sem_replica_groups.extend(
                set(g) for g in groups)

            def defer_wait(bi, sem, val):
                bi._wait_ge(sem, 0)
                deferred_waits.append((bi, sem, val))
                return bi

            for i, s in enumerate(rsem + [lsem]):
                cl = gp_order(nc.gpsimd.sem_clear(s))
                if i == 0:
                    defer_wait(cl, nc._bir_kernel_barrier_sem,
                               nc.bir_kernel_barrier_sem_inc)

            # ---- persistent tensors ----
            x_pad = [[const.tile([128, PLANE], bf16, tag=f"xp{j}_{n}",
                                 name=f"xp{j}_{n}")
                      for n in range(NL)] for j in range(2)]
            h1m = [const.tile([128, PLANE], bf16, tag=f"h1m{n}", name=f"h1m{n}")
                   for n in range(NL)]
            h1o = const.tile([B1, 2 * GUARD + NL * PLANE], bf16, tag="h1o",
                             name="h1o")
            h2m = [const.tile([128, HW], bf16, tag=f"h2m{n}", name=f"h2m{n}")
                   for n in range(NL)]
            h2o = const.tile([B2, NL * HW], bf16, tag="h2o", name="h2o")

            def h1o_img(n):
                return h1o[:, GUARD + n * PLANE:GUARD + (n + 1) * PLANE]

            def zero_pads(t):
                nc.vector.memset(t[:, 0:STRIP0], 0.0)
                pairs = t[:, 2 * PW - 1:2 * PW - 1 + 56 * PW].rearrange(
                    "p (r c) -> p r c", c=PW)[:, :, 0:2]
                nc.vector.memset(pairs, 0.0)
                nc.vector.memset(t[:, STRIP0 + 56 * PW:PLANE], 0.0)

            for j in range(2):
                for n in range(NL):
                    zero_pads(x_pad[j][n])
            for n in range(NL):
                zero_pads(h1m[n])
            nc.vector.memset(h1o[:, 0:GUARD], 0.0)
            nc.vector.memset(h1o[:, GUARD + NL * PLANE:], 0.0)
            for n in range(NL):
                zero_pads(h1o_img(n))

            accs = {}
            for nm, rows in (("A1", 128), ("B1", B1), ("A2", 128), ("B2", B2)):
                for s in ("s", "q"):
                    accs[(nm, s)] = const.tile([rows, NL * NCHUNK], f32,
                                               tag=f"ac{nm}{s}",
                                               name=f"ac{nm}{s}")

            # ---- head: stream x in, cast into padded bf16 planes ----
            for n in range(NL):
                for j in range(2):
                    for q in range(4):
                        r0 = q * QUAD_ROWS
                        xs = otp.tile([128, QUAD_ELEMS], f32, tag="xs",
                                      name="xs")
                        nc.sync.dma_start(
                            xs[:],
                            x_d[n, j * 128:(j + 1) * 128, r0:r0 + QUAD_ROWS, :])
                        dst = interior(x_pad[j][n], (r0 + 1) * PW + 1,
                                       QUAD_ROWS)
                        nc.vector.tensor_copy(
                            dst, xs[:, :].rearrange("p (r c) -> p r c", c=W))

            # ---- generic 128-row conv group (chunked, stats via ACT) ----
            def conv_a(mms_fn, n, acc_nm, dst_fn):
                last = None
                for k in range(NCHUNK):
                    pt = psA.tile([128, 8 * W], f32, tag="ps", name="ps")
                    mms = mms_fn(n, k)
                    for idx, (lh, rhs) in enumerate(mms):
                        nc.tensor.matmul(pt[:], lh, rhs, start=(idx == 0),
                                         stop=(idx == len(mms) - 1))
                    src_int = pt[:, 0:8 * W].rearrange("p (r c) -> p r c", c=W)
                    col = n * NCHUNK + k
                    dst_int = dst_fn(n, k)
                    nc.scalar.activation(
                        dst_int, src_int, AF.Copy,
                        accum_out=accs[(acc_nm, "s")][:, col:col + 1])
                    sq = sqp.tile([128, 8 * W], f32, tag="sq", name="sq")
                    last = nc.scalar.activation(
                        sq[:, :].rearrange("p (r c) -> p r c", c=W),
                        dst_int, AF.Square,
                        accum_out=accs[(acc_nm, "q")][:, col:col + 1])
                return last

            def c1a_mms(n, k):
                out = []
                for j in range(2):
                    for t in range(9):
                        ty, tx = t // 3, t % 3
                        dq = (ty - 1) * PW + (tx - 1)
                        off = STRIP0 + CHUNK * k + dq
                        rhs = x_pad[j][n][:, off:off + CHUNK].rearrange(
                            "p (r c) -> p r c", c=PW)[:, :, 0:W]
                        out.append((wcol(f"c1m_{j}_{t}", 128, 128), rhs))
                return out

            def h1m_dst(n, k):
                return interior(h1m[n], (1 + 8 * k) * PW + 1, 8)

            for n in range(NL):
                conv_a(c1a_mms, n, "A1", h1m_dst)

            # ---- exchange send/recv ----
            ex_rv = [const.tile([128, 16], f32, tag=f"rv{e}", name=f"rv{e}")
                     for e in range(4)]
            ex_pk = [const.tile([128, 2], f32, tag=f"pk{e}", name=f"pk{e}")
                     for e in range(4)]

            def ex_send(e, acc_nm, rows, after=None):
                pk = ex_pk[e]
                if rows < 128:
                    nc.vector.memset(pk[:], 0.0)
                r1 = nc.vector.tensor_reduce(
                    pk[0:rows, 0:1], accs[(acc_nm, "s")][:], axis=AX.X,
                    op=ALU.add)
                if after is not None:
                    bass_mod._add_dep_helper(r1.ins, after.ins, sync=True,
                                             reason="send ordering")
                nc.vector.tensor_reduce(
                    pk[0:rows, 1:2], accs[(acc_nm, "q")][:], axis=AX.X,
                    op=ALU.add)
                cp = nc.vector.tensor_copy(ex_rv[e][:, 0:2], pk[:])
                for d in range(1, 8):
                    rd = [None] * 8
                    rd[d] = (0, d)
                    gp_order(nc.gpsimd.remote_dma_broadcast(
                        ex_rv[e][:, 2 * d:2 * d + 2], pk[:],
                        remote_sem=rsem[e], local_sem=lsem, rdests=rd,
                        queue_num=(d - 1) % 4))
                for q in range(4):
                    gp_order(nc.gpsimd.trigger_dma(count=None, queue_num=q))
                return cp

            def ex_recv(e, rows, after=None):
                gl = const.tile([128, 2], f32, tag=f"gl{e}", name=f"gl{e}")
                red = nc.vector.tensor_reduce(
                    gl[0:rows, :],
                    ex_rv[e][0:rows, 0:16].rearrange("p (s c) -> p c s", c=2),
                    axis=AX.X, op=ALU.add)
                defer_wait(red, rsem[e], 14)
                if after is not None:
                    bass_mod._add_dep_helper(red.ins, after.ins, sync=True,
                                             reason="recv after phase")
                return gl

            def bn_affine(gl, rows, g_ap, b_ap, sfx):
                def t1(tag):
                    return const.tile([rows, 1], f32, tag=f"{tag}{sfx}",
                                      name=f"{tag}{sfx}")
                mean, var, y, vh, tmp = (t1(x) for x in
                                         ("mn", "vr", "y", "vh", "tm"))
                nc.vector.tensor_scalar_mul(mean[:], gl[0:rows, 0:1],
                                            1.0 / COUNT)
                nc.vector.tensor_tensor(var[:], mean[:], mean[:], ALU.mult)
                nc.vector.scalar_tensor_tensor(
                    var[:], gl[0:rows, 1:2], 1.0 / COUNT, var[:],
                    ALU.mult, ALU.subtract)
                nc.vector.tensor_scalar_add(var[:], var[:], EPS)
                iv = var[:].bitcast(mybir.dt.int32)
                yi = y[:].bitcast(mybir.dt.int32)
                nc.vector.tensor_scalar(yi, iv, 1, None, ALU.arith_shift_right)
                nc.vector.tensor_scalar(yi, yi, -1, None, ALU.bitwise_xor)
                nc.vector.tensor_scalar(yi, yi, 0x5f3759df + 1, None, ALU.add)
                nc.vector.tensor_scalar_mul(vh[:], var[:], 0.5)
                for _ in range(2):
                    nc.vector.tensor_tensor(tmp[:], y[:], y[:], ALU.mult)
                    nc.vector.tensor_tensor(tmp[:], tmp[:], vh[:], ALU.mult)
                    nc.vector.tensor_scalar(tmp[:], tmp[:], -1.0, 1.5,
                                            ALU.mult, ALU.add)
                    nc.vector.tensor_tensor(y[:], y[:], tmp[:], ALU.mult)
                sc = t1("sc")
                nc.vector.tensor_tensor(sc[:], g_ap, y[:], ALU.mult)
                bi = t1("bi")
                nc.vector.tensor_tensor(bi[:], mean[:], sc[:], ALU.mult)
                nc.vector.tensor_tensor(bi[:], b_ap, bi[:], ALU.subtract)
                return sc, bi

            ex_send(0, "A1", 128)

            # ---- conv1 overflow: M-packed + PE recomb ----
            c1b_last = None
            for n in range(NL):
                sb = sbp.tile([NB1, PLANE], bf16, tag="sb81", name="sb81")
                for k in range(NBCH):
                    c0 = CHUNK * k
                    F = min(CHUNK, PLANE - c0)
                    pt = psB.tile([NB1, CHUNK], f32, tag="psB", name="psB")
                    for j in range(2):
                        nc.tensor.matmul(
                            pt[0:NB1, 0:F], wcol(f"c1o_{j}", 128, NB1),
                            x_pad[j][n][:, c0:c0 + F],
                            start=(j == 0), stop=(j == 1))
                    nc.scalar.activation(sb[:, c0:c0 + F], pt[0:NB1, 0:F],
                                         AF.Copy)
                for k in range(NCHUNK):
                    pt = psR.tile([B1, 8 * W], f32, tag="psR", name="psR")
                    for t in range(9):
                        ty, tx = t // 3, t % 3
                        dq = (ty - 1) * PW + (tx - 1)
                        off = STRIP0 + CHUNK * k + dq
                        rhs = sb[0:NB1, off:off + CHUNK].rearrange(
                            "p (r c) -> p r c", c=PW)[:, :, 0:W]
                        nc.tensor.matmul(pt[:], wcol(f"sel_{t}", NB1, B1),
                                         rhs, start=(t == 0), stop=(t == 8))
                    src_int = pt[:, 0:8 * W].rearrange("p (r c) -> p r c", c=W)
                    dst_int = interior(h1o_img(n), (1 + 8 * k) * PW + 1, 8)
                    col = n * NCHUNK + k
                    nc.scalar.activation(
                        dst_int, src_int, AF.Copy,
                        accum_out=accs[("B1", "s")][:, col:col + 1])
                    sq = sqp.tile([128, 8 * W], f32, tag="sq", name="sq")
                    c1b_last = nc.scalar.activation(
                        sq[0:B1, :].rearrange("p (r c) -> p r c", c=W),
                        dst_int, AF.Square,
                        accum_out=accs[("B1", "q")][:, col:col + 1])

            # ---- masked-out tail: out = relu(x + beta2) (no BN2 dep) ----
            for n in range(NL):
                for rh in range(2):
                    r0 = rh * HALF_ROWS
                    xv = interior(x_pad[1][n], (r0 + 1) * PW + 1, HALF_ROWS)
                    ot = otp.tile([128, HALF_ELEMS], f32, tag="ot", name="ot")
                    otv = ot[:, :].rearrange("p (r c) -> p r c", c=W)
                    mt_last = nc.vector.tensor_scalar(
                        otv, xv, aff_sb[:, 8:9], 0.0, ALU.add, ALU.max)
                    nc.sync.dma_start(
                        out_d[n, 128 + B2:C, r0:r0 + HALF_ROWS, :],
                        ot[B2:128, :])

            sB1 = ex_send(1, "B1", B1, after=mt_last)

            # ---- BN1 main: recv, affine, apply.  The recv/affine (DVE)
            # may run as soon as stats arrive, but the ACT applies are
            # pinned after conv1B's last evac so the scheduler cannot
            # place them ahead of conv1B's ACT stream. ----
            gl = ex_recv(0, 128, sB1)
            s1m, b1m = bn_affine(gl, 128, aff_sb[:, 0:1], aff_sb[:, 1:2], "1m")
            apA_last = None
            for n in range(NL):
                v = interior(h1m[n], STRIP0, H)
                apA_last = nc.scalar.activation(v, v, AF.Relu, bias=b1m[:],
                                                scale=s1m[:])
                bass_mod._add_dep_helper(apA_last.ins, c1b_last.ins,
                                         sync=True,
                                         reason="applyA after conv1B")

            # ---- BN1 ovf: recv, affine (pinned after applyA so the
            # blocked recv cannot split the affineA->applyA chain) ----
            glb = ex_recv(1, B1, apA_last)
            s1o, b1o = bn_affine(glb, B1, aff_sb[0:B1, 2:3],
                                 aff_sb[0:B1, 3:4], "1o")

            # presh: tap-shifted replicas of the (post-BN) overflow planes
            presh = {}

            def replicate(n):
                pr = prp.tile([NB1, PLANE], bf16, tag="pr", name="pr")
                presh[n] = pr
                for t in range(9):
                    ty, tx = t // 3, t % 3
                    dq = (ty - 1) * PW + (tx - 1)
                    src = h1o[0:B1, GUARD + n * PLANE + dq:
                              GUARD + n * PLANE + dq + PLANE]
                    nc.sync.dma_start(pr[9 * t:9 * t + B1, :], src)

            # ---- conv2 ----
            def c2_mms(n, k, nm_main, nm_ovf, co):
                out = []
                for t in range(9):
                    ty, tx = t // 3, t % 3
                    dq = (ty - 1) * PW + (tx - 1)
                    off = STRIP0 + CHUNK * k + dq
                    rhs = h1m[n][:, off:off + CHUNK].rearrange(
                        "p (r c) -> p r c", c=PW)[:, :, 0:W]
                    out.append((wcol(f"{nm_main}_{t}", 128, co), rhs))
                if nm_ovf is not None:
                    off = STRIP0 + CHUNK * k
                    rhs = presh[n][:, off:off + CHUNK].rearrange(
                        "p (r c) -> p r c", c=PW)[:, :, 0:W]
                    out.append((wcol(nm_ovf, NB1, co), rhs))
                return out

            def h2m_dst(n, k):
                return h2m[n][:, 8 * k * W:(8 * k + 8) * W].rearrange(
                    "p (r c) -> p r c", c=W)

            # conv2 main group, main-ci only (no stats yet): the overflow
            # contribution is added afterwards so no part of conv2A waits
            # on the BN1-ovf exchange.
            main_last = None
            for n in range(NL):
                for k in range(NCHUNK):
                    pt = psA.tile([128, 8 * W], f32, tag="ps", name="ps")
                    mms = c2_mms(n, k, "c2m", None, 128)
                    for idx, (lh, rhs) in enumerate(mms):
                        nc.tensor.matmul(pt[:], lh, rhs, start=(idx == 0),
                                         stop=(idx == len(mms) - 1))
                    main_last = nc.scalar.activation(
                        h2m_dst(n, k),
                        pt[:, 0:8 * W].rearrange("p (r c) -> p r c", c=W),
                        AF.Copy)

            # BN1-ovf apply (pinned after the conv2A-main evacs so the
            # scheduler cannot stall them on the exchange) + replication
            for n in range(NL):
                v = interior(h1o_img(n), STRIP0, H)
                ap = nc.scalar.activation(v, v, AF.Relu, bias=b1o[:],
                                          scale=s1o[:])
                bass_mod._add_dep_helper(ap.ins, main_last.ins, sync=True,
                                         reason="applyB after conv2A main")
            for n in range(NL):
                replicate(n)

            # overflow-ci contribution + stats for the conv2 main group
            a2_last = None
            for n in range(NL):
                for k in range(NCHUNK):
                    pt = psA.tile([128, 8 * W], f32, tag="ps", name="ps")
                    off = STRIP0 + CHUNK * k
                    rhs = presh[n][:, off:off + CHUNK].rearrange(
                        "p (r c) -> p r c", c=PW)[:, :, 0:W]
                    nc.tensor.matmul(pt[:], wcol("c2mo", NB1, 128), rhs,
                                     start=True, stop=True)
                    ob = sqp.tile([128, 8 * W], bf16, tag="sqb", name="sqb")
                    obv = ob[:, :].rearrange("p (r c) -> p r c", c=W)
                    nc.scalar.activation(
                        obv, pt[:, 0:8 * W].rearrange("p (r c) -> p r c",
                                                      c=W), AF.Copy)
                    h2v = h2m_dst(n, k)
                    nc.vector.tensor_tensor(h2v, h2v, obv, ALU.add)
                    col = n * NCHUNK + k
                    sq = sqp.tile([128, 8 * W], f32, tag="sq", name="sq")
                    sqv = sq[:, :].rearrange("p (r c) -> p r c", c=W)
                    nc.scalar.activation(
                        sqv, h2v, AF.Copy,
                        accum_out=accs[("A2", "s")][:, col:col + 1])
                    a2_last = nc.scalar.activation(
                        sqv, h2v, AF.Square,
                        accum_out=accs[("A2", "q")][:, col:col + 1])

            ex_send(2, "A2", 128)

            # conv2 overflow output group (M=B2)
            c2b_last = None
            for n in range(NL):
                replicate(n)
                for k in range(NCHUNK):
                    pt = psR.tile([B1, 8 * W], f32, tag="psR", name="psR")
                    mms = c2_mms(n, k, "c2b", "c2bo", B2)
                    for idx, (lh, rhs) in enumerate(mms):
                        nc.tensor.matmul(pt[0:B2, :], lh, rhs,
                                         start=(idx == 0),
                                         stop=(idx == len(mms) - 1))
                    src_int = pt[0:B2, 0:8 * W].rearrange(
                        "p (r c) -> p r c", c=W)
                    col = n * NCHUNK + k
                    dst = h2o[0:B2, n * HW + 8 * k * W:
                              n * HW + (8 * k + 8) * W].rearrange(
                        "p (r c) -> p r c", c=W)
                    nc.scalar.activation(
                        dst, src_int, AF.Copy,
                        accum_out=accs[("B2", "s")][:, col:col + 1])
                    sq = sqp.tile([128, 8 * W], f32, tag="sq", name="sq")
                    c2b_last = nc.scalar.activation(
                        sq[0:B2, :].rearrange("p (r c) -> p r c", c=W),
                        dst, AF.Square,
                        accum_out=accs[("B2", "q")][:, col:col + 1])

            # B2 stats go out as soon as conv2B finishes; the tail-A chain
            # below then runs during the B2 exchange flight.  recvA2 is
            # pinned after the B2 send so the scheduler cannot float the
            # B2 reduces past the tail-A chain.
            sB2 = ex_send(3, "B2", B2)

            # ---- BN2 main: recv, affine, tail (128 kept, all-DVE) ----
            gl2 = ex_recv(2, 128, sB2)
            s2m, b2m = bn_affine(gl2, 128, aff_sb[:, 4:5], aff_sb[:, 5:6],
                                 "2m")
            tailA_last = None
            for n in range(NL):
                for rh in range(2):
                    r0 = rh * HALF_ROWS
                    xv = interior(x_pad[0][n], (r0 + 1) * PW + 1, HALF_ROWS)
                    h2v = h2m[n][:, r0 * W:r0 * W + HALF_ELEMS].rearrange(
                        "p (r c) -> p r c", c=W)
                    ot = otp.tile([128, HALF_ELEMS], f32, tag="ot", name="ot")
                    otv = ot[:, :].rearrange("p (r c) -> p r c", c=W)
                    nc.vector.scalar_tensor_tensor(
                        otv, h2v, s2m[:], xv, ALU.mult, ALU.add)
                    tailA_last = nc.vector.tensor_scalar(
                        ot[:], ot[:], b2m[:], 0.0, ALU.add, ALU.max)
                    nc.sync.dma_start(
                        out_d[n, 0:128, r0:r0 + HALF_ROWS, :], ot[:])

            # ---- BN2 ovf: recv (pinned after tail-A so the blocked recv
            # cannot split the affineA2->tailA chain), affine, tail ----
            gl2o = ex_recv(3, B2, tailA_last)
            s2o, b2o = bn_affine(gl2o, B2, aff_sb[0:B2, 6:7],
                                 aff_sb[0:B2, 7:8], "2o")
            for n in range(NL):
                for rh in range(2):
                    r0 = rh * HALF_ROWS
                    xv = interior(x_pad[1][n], (r0 + 1) * PW + 1, HALF_ROWS)
                    h2v = h2o[0:B2, n * HW + r0 * W:
                              n * HW + r0 * W + HALF_ELEMS].rearrange(
                        "p (r c) -> p r c", c=W)
                    ot = otp.tile([128, HALF_ELEMS], f32, tag="ot", name="ot")
                    otv = ot[:, :].rearrange("p (r c) -> p r c", c=W)
                    nc.vector.scalar_tensor_tensor(
                        otv[0:B2], h2v, s2o[:], xv[0:B2], ALU.mult, ALU.add)
                    nc.vector.tensor_scalar(ot[0:B2, :], ot[0:B2, :],
                                            b2o[:], 0.0, ALU.add, ALU.max)
                    nc.sync.dma_start(
                        out_d[n, 128:128 + B2, r0:r0 + HALF_ROWS, :],
                        ot[0:B2, :])

    for bi, sem, val in deferred_waits:
        patched = False
        for w in bi.ins.sync_info.on_wait:
            if w.id == sem.num and w.wait_value == 0:
                w.wait_value = val
                patched = True
                break
        assert patched, f"deferred wait not found on {bi.ins.name}"

    nc.compile()
    return nc


def kernel(x, W1, W2, gamma1, beta1, gamma2, beta2, mask1, mask2,
           _trace=False, _trace_kwargs=None):
    from concourse.bass_utils import run_bass_kernel_spmd

    wt, aff, plan, offs = _pack(W1, W2, gamma1, beta1, gamma2, beta2,
                                mask1, mask2)
    k1m, k1o, k2m, k2o, masked2, P = plan

    key = (len(k1o), len(k2o), wt.shape[1])
    if _cache.get("key") != key:
        _cache["nc"] = _build(len(k1o), len(k2o), offs, wt.shape[1])
        _cache["key"] = key
    nc = _cache["nc"]

    x = np.ascontiguousarray(np.asarray(x, np.float32)[:, P])

    in_maps = [{"x": x[i * NL:(i + 1) * NL], "wt": wt, "aff": aff}
               for i in range(N_CORES)]
    kw = {}
    if _trace:
        kw = dict(trace=True, **(_trace_kwargs or {}))
    res = run_bass_kernel_spmd(nc, in_maps, core_ids=list(range(N_CORES)),
                               **kw)
    out_p = np.concatenate([res.results[i]["out"] for i in range(N_CORES)],
                           axis=0)
    _cache["last_results"] = res
    inv = np.empty(C, np.int64)
    inv[P] = np.arange(C)
    return np.ascontiguousarray(out_p[:, inv])


# revision 3
# speedup vs baseline: 1.0768x; 1.0516x over previous
"""Trainium2 Bass kernel for a ResNet BasicBlock (dense CNN, sync-BN).

Reference computation (training-mode BN, batch stats over (N,H,W)):
    h = conv3x3(x, W1) * mask1            # structured channel pruning
    h = relu(bn(h, gamma1, beta1))
    h = conv3x3(h, W2) * mask2
    h = bn(h, gamma2, beta2)
    out = relu(h + x)                      # identity shortcut

v2 exploits the mask sparsity: only K1=|mask1| conv1 outputs and
K2=|mask2| conv2 outputs are live (masked channels are exactly zero
through the block since beta=0, and masked conv2 outputs reduce to
relu(x + beta2)).

  - conv1: 128-channel main group (direct conv, 18 mm/chunk) + overflow
    group (K1-128 channels) M-packed: 2 matmuls per contiguous 464-col
    chunk produce per-(co,tap) partials for all 9 taps at once, then 9
    identity matmuls per interior chunk recombine them with tap-shifted
    reads (zero-padding falls out of the zero-padded planes).
  - conv2 contracts over only K1 inputs: 9 matmuls over the 128 main
    channels plus ONE K=9*(K1-128) matmul over pre-shifted replicas of
    the overflow planes, for both the 128-wide main output group and the
    (K2-128)-wide overflow group: 20 mm/chunk vs 36 unpruned.
  - channels host-permuted so conv2-kept channels sit in contiguous
    partition blocks; out is written permuted, un-permuted on the host.
    Masked outputs (relu(x+beta2)) have no BN2 dep: computed mid-kernel.

Sync-BN: 4 tiny XOR-relative remote-DMA all-gathers (one per channel
group), each sent right after its group's convs so the transfer hides
under remaining conv work; the 7 broadcasts per exchange spread over 4
SWDGE queues to shorten the descriptor burst.
"""

import numpy as np
import ml_dtypes

N_TOT, C, H, W = 32, 256, 56, 56
N_CORES = 8
NL = N_TOT // N_CORES
PW = H + 2                     # 58
PLANE = PW * PW + 4            # 3368
GUARD = 64
STRIP0 = PW + 1                # 59
CHUNK = 8 * PW                 # 464
NCHUNK = 7
NBCH = 8                       # ceil(PLANE/CHUNK): contiguous chunks
HW = H * W                     # 3136
HALF_ROWS = 28
HALF_ELEMS = HALF_ROWS * W
QUAD_ROWS = 14
QUAD_ELEMS = QUAD_ROWS * W
COUNT = N_TOT * HW
EPS = 1e-5

_BF16 = ml_dtypes.bfloat16
_cache = {}


def _plan(mask1, mask2, beta1):
    m1 = np.asarray(mask1) != 0
    live1 = m1 | (np.maximum(np.asarray(beta1), 0.0) > 0)
    kept1 = np.where(live1)[0]
    kept2 = np.where(np.asarray(mask2) != 0)[0]
    masked2 = np.where(np.asarray(mask2) == 0)[0]
    assert 128 < len(kept1) <= 128 + 14, f"K1={len(kept1)} unsupported"
    assert 128 < len(kept2) <= 128 + 14, f"K2={len(kept2)} unsupported"
    k1m, k1o = kept1[:128], kept1[128:]
    k2m, k2o = kept2[:128], kept2[128:]
    P = np.concatenate([k2m, k2o, masked2])
    assert len(P) == C
    return k1m, k1o, k2m, k2o, masked2, P


def _pack(W1, W2, gamma1, beta1, gamma2, beta2, mask1, mask2):
    k1m, k1o, k2m, k2o, masked2, P = _plan(mask1, mask2, beta1)
    B1, B2 = len(k1o), len(k2o)
    W1 = np.asarray(W1, np.float32) * (np.asarray(mask1, np.float32) != 0)[:, None, None, None]
    W2 = np.asarray(W2, np.float32) * (np.asarray(mask2, np.float32) != 0)[:, None, None, None]

    cols, offs = [], {}

    def add(name, block):
        r, c = block.shape
        b = np.zeros((128, c), np.float32)
        b[:r] = block
        offs[name] = sum(x.shape[1] for x in cols)
        cols.append(b)

    for j in range(2):
        for t in range(9):
            ty, tx = t // 3, t % 3
            add(f"c1m_{j}_{t}",
                W1[np.ix_(k1m, P[j * 128:(j + 1) * 128])][:, :, ty, tx].T)
    for j in range(2):
        blk = np.zeros((128, 9 * B1), np.float32)
        for t in range(9):
            ty, tx = t // 3, t % 3
            for c in range(B1):
                blk[:, 9 * t + c] = W1[k1o[c], P[j * 128:(j + 1) * 128], ty, tx]
        add(f"c1o_{j}", blk)
    # tap-selector blocks for the recomb matmuls: sel_t [9*B1, B1] picks
    # rows (c,t)=9t+c of the packed partials (rhs must start at partition 0)
    for t in range(9):
        blk = np.zeros((9 * B1, B1), np.float32)
        for c in range(B1):
            blk[9 * t + c, c] = 1.0
        add(f"sel_{t}", blk)
    for t in range(9):
        ty, tx = t // 3, t % 3
        add(f"c2m_{t}", W2[np.ix_(k2m, k1m)][:, :, ty, tx].T)
    blk = np.zeros((9 * B1, 128), np.float32)
    for t in range(9):
        ty, tx = t // 3, t % 3
        for c in range(B1):
            blk[9 * t + c, :] = W2[k2m, k1o[c], ty, tx]
    add("c2mo", blk)
    for t in range(9):
        ty, tx = t // 3, t % 3
        add(f"c2b_{t}", W2[np.ix_(k2o, k1m)][:, :, ty, tx].T)
    blk = np.zeros((9 * B1, B2), np.float32)
    for t in range(9):
        ty, tx = t // 3, t % 3
        for c in range(B1):
            blk[9 * t + c, :] = W2[k2o, k1o[c], ty, tx]
    add("c2bo", blk)

    wt = np.concatenate(cols, axis=1).astype(_BF16)

    aff = np.zeros((128, 16), np.float32)
    g1, b1 = np.asarray(gamma1, np.float32), np.asarray(beta1, np.float32)
    g2, b2 = np.asarray(gamma2, np.float32), np.asarray(beta2, np.float32)
    aff[:, 0], aff[:, 1] = g1[k1m], b1[k1m]
    aff[:B1, 2], aff[:B1, 3] = g1[k1o], b1[k1o]
    aff[:, 4], aff[:, 5] = g2[k2m], b2[k2m]
    aff[:B2, 6], aff[:B2, 7] = g2[k2o], b2[k2o]
    aff[:B2, 8] = b2[k2o]              # rows 0:B2 computed but never output
    aff[B2:, 8] = b2[masked2]          # aligned with x half-1 rows B2:128
    return wt, aff, (k1m, k1o, k2m, k2o, masked2, P), offs


def _build(B1, B2, offs, ncol):
    import concourse.bass as bass_mod
    import concourse.bacc as bacc
    import concourse.mybir as mybir
    import concourse.tile as tile

    f32 = mybir.dt.float32
    bf16 = mybir.dt.bfloat16
    AX = mybir.AxisListType
    ALU = mybir.AluOpType
    AF = mybir.ActivationFunctionType

    NB1 = 9 * B1
    nc = bacc.Bacc("TRN2", target_bir_lowering=False, debug=False,
                   num_devices=N_CORES, num_swdge_queues=4)

    x_d = nc.dram_tensor("x", [NL, C, H, W], f32, kind="ExternalInput")
    wt_d = nc.dram_tensor("wt", [128, ncol], bf16, kind="ExternalInput")
    aff_d = nc.dram_tensor("aff", [128, 16], f32, kind="ExternalInput")
    out_d = nc.dram_tensor("out", [NL, C, H, W], f32, kind="ExternalOutput")

    groups = [list(range(N_CORES))]

    def interior(t, base, nrows):
        v = t[:, base:base + nrows * PW].rearrange("p (r c) -> p r c", c=PW)
        return v[:, :, 0:W]

    with tile.TileContext(nc) as tc:
        import contextlib
        with contextlib.ExitStack() as ctx:
            const = ctx.enter_context(tc.tile_pool(name="const", bufs=1))
            psA = ctx.enter_context(tc.tile_pool(name="psA", bufs=5, space="PSUM"))
            psB = ctx.enter_context(tc.tile_pool(name="psB", bufs=1, space="PSUM"))
            psR = ctx.enter_context(tc.tile_pool(name="psR", bufs=2, space="PSUM"))
            otp = ctx.enter_context(tc.tile_pool(name="otp", bufs=2))
            sqp = ctx.enter_context(tc.tile_pool(name="sqp", bufs=1))
            sbp = ctx.enter_context(tc.tile_pool(name="sbp", bufs=1))
            prp = ctx.enter_context(tc.tile_pool(name="prp", bufs=2))

            wt_sb = const.tile([128, ncol], bf16, tag="wt", name="wt")
            nc.sync.dma_start(wt_sb[:], wt_d[:, :])
            aff_sb = const.tile([128, 16], f32, tag="aff", name="aff")
            nc.sync.dma_start(aff_sb[:], aff_d[:])

            def wcol(name, r, cw):
                o = offs[name]
                return wt_sb[0:r, o:o + cw]

            # ---- cross-core stats exchange plumbing ----
            rsem = [nc.alloc_semaphore(f"rst{i}") for i in range(4)]
            lsem = nc.alloc_semaphore("lst")
            _gp_prev = [None]
            deferred_waits = []

            def gp_order(bi):
                if _gp_prev[0] is not None:
                    bass_mod._add_dep_helper(bi.ins, _gp_prev[0].ins,
                                             sync=False, reason="stats order")
                _gp_prev[0] = bi
                return bi

            nc._bir_kernel_barrier_sem_replica_groups.extend(
                set(g) for g in groups)

            def defer_wait(bi, sem, val):
                bi._wait_ge(sem, 0)
                deferred_waits.append((bi, sem, val))
                return bi

            for i, s in enumerate(rsem + [lsem]):
                cl = gp_order(nc.gpsimd.sem_clear(s))
                if i == 0:
                    defer_wait(cl, nc._bir_kernel_barrier_sem,
                               nc.bir_kernel_barrier_sem_inc)

            # ---- persistent tensors ----
            x_pad = [[const.tile([128, PLANE], bf16, tag=f"xp{j}_{n}",
                                 name=f"xp{j}_{n}")
                      for n in range(NL)] for j in range(2)]
            h1m = [const.tile([128, PLANE], bf16, tag=f"h1m{n}", name=f"h1m{n}")
                   for n in range(NL)]
            h1o = const.tile([B1, 2 * GUARD + NL * PLANE], bf16, tag="h1o",
                             name="h1o")
            h2m = [const.tile([128, HW], bf16, tag=f"h2m{n}", name=f"h2m{n}")
                   for n in range(NL)]
            h2o = const.tile([B2, NL * HW], bf16, tag="h2o", name="h2o")

            def h1o_img(n):
                return h1o[:, GUARD + n * PLANE:GUARD + (n + 1) * PLANE]

            def zero_pads(t):
                nc.vector.memset(t[:, 0:STRIP0], 0.0)
                pairs = t[:, 2 * PW - 1:2 * PW - 1 + 56 * PW].rearrange(
                    "p (r c) -> p r c", c=PW)[:, :, 0:2]
                nc.vector.memset(pairs, 0.0)
                nc.vector.memset(t[:, STRIP0 + 56 * PW:PLANE], 0.0)

            for j in range(2):
                for n in range(NL):
                    zero_pads(x_pad[j][n])
            for n in range(NL):
                zero_pads(h1m[n])
            nc.vector.memset(h1o[:, 0:GUARD], 0.0)
            nc.vector.memset(h1o[:, GUARD + NL * PLANE:], 0.0)
            for n in range(NL):
                zero_pads(h1o_img(n))

            accs = {}
            for nm, rows in (("A1", 128), ("B1", B1), ("A2", 128), ("B2", B2)):
                for s in ("s", "q"):
                    accs[(nm, s)] = const.tile([rows, NL * NCHUNK], f32,
                                               tag=f"ac{nm}{s}",
                                               name=f"ac{nm}{s}")

            # ---- head: stream x in, cast into padded bf16 planes ----
            for n in range(NL):
                for j in range(2):
                    for q in range(4):
                        r0 = q * QUAD_ROWS
                        xs = otp.tile([128, QUAD_ELEMS], f32, tag="xs",
                                      name="xs")
                        nc.sync.dma_start(
                            xs[:],
                            x_d[n, j * 128:(j + 1) * 128, r0:r0 + QUAD_ROWS, :])
                        dst = interior(x_pad[j][n], (r0 + 1) * PW + 1,
                                       QUAD_ROWS)
                        nc.vector.tensor_copy(
                            dst, xs[:, :].rearrange("p (r c) -> p r c", c=W))

            # ---- generic 128-row conv group (chunked, stats via ACT) ----
            def conv_a(mms_fn, n, acc_nm, dst_fn):
                last = None
                for k in range(NCHUNK):
                    pt = psA.tile([128, 8 * W], f32, tag="ps", name="ps")
                    mms = mms_fn(n, k)
                    for idx, (lh, rhs) in enumerate(mms):
                        nc.tensor.matmul(pt[:], lh, rhs, start=(idx == 0),
                                         stop=(idx == len(mms) - 1))
                    src_int = pt[:, 0:8 * W].rearrange("p (r c) -> p r c", c=W)
                    col = n * NCHUNK + k
                    dst_int = dst_fn(n, k)
                    nc.scalar.activation(
                        dst_int, src_int, AF.Copy,
                        accum_out=accs[(acc_nm, "s")][:, col:col + 1])
                    sq = sqp.tile([128, 8 * W], f32, tag="sq", name="sq")
                    last = nc.scalar.activation(
                        sq[:, :].rearrange("p (r c) -> p r c", c=W),
                        dst_int, AF.Square,
                        accum_out=accs[(acc_nm, "q")][:, col:col + 1])
                return last

            def c1a_mms(n, k):
                out = []
                for j in range(2):
                    for t in range(9):
                        ty, tx = t // 3, t % 3
                        dq = (ty - 1) * PW + (tx - 1)
                        off = STRIP0 + CHUNK * k + dq
                        rhs = x_pad[j][n][:, off:off + CHUNK].rearrange(
                            "p (r c) -> p r c", c=PW)[:, :, 0:W]
                        out.append((wcol(f"c1m_{j}_{t}", 128, 128), rhs))
                return out

            def h1m_dst(n, k):
                return interior(h1m[n], (1 + 8 * k) * PW + 1, 8)

            for n in range(NL):
                conv_a(c1a_mms, n, "A1", h1m_dst)

            # ---- exchange send/recv ----
            ex_rv = [const.tile([128, 16], f32, tag=f"rv{e}", name=f"rv{e}")
                     for e in range(4)]
            ex_pk = [const.tile([128, 2], f32, tag=f"pk{e}", name=f"pk{e}")
                     for e in range(4)]

            def ex_send(e, acc_nm, rows, after=None):
                pk = ex_pk[e]
                if rows < 128:
                    nc.vector.memset(pk[:], 0.0)
                r1 = nc.vector.tensor_reduce(
                    pk[0:rows, 0:1], accs[(acc_nm, "s")][:], axis=AX.X,
                    op=ALU.add)
                if after is not None:
                    bass_mod._add_dep_helper(r1.ins, after.ins, sync=True,
                                             reason="send ordering")
                nc.vector.tensor_reduce(
                    pk[0:rows, 1:2], accs[(acc_nm, "q")][:], axis=AX.X,
                    op=ALU.add)
                cp = nc.vector.tensor_copy(ex_rv[e][:, 0:2], pk[:])
                for d in range(1, 8):
                    rd = [None] * 8
                    rd[d] = (0, d)
                    gp_order(nc.gpsimd.remote_dma_broadcast(
                        ex_rv[e][:, 2 * d:2 * d + 2], pk[:],
                        remote_sem=rsem[e], local_sem=lsem, rdests=rd,
                        queue_num=(d - 1) % 4))
                for q in range(4):
                    gp_order(nc.gpsimd.trigger_dma(count=None, queue_num=q))
                return cp

            def ex_recv(e, rows, after=None):
                gl = const.tile([128, 2], f32, tag=f"gl{e}", name=f"gl{e}")
                red = nc.vector.tensor_reduce(
                    gl[0:rows, :],
                    ex_rv[e][0:rows, 0:16].rearrange("p (s c) -> p c s", c=2),
                    axis=AX.X, op=ALU.add)
                defer_wait(red, rsem[e], 14)
                if after is not None:
                    bass_mod._add_dep_helper(red.ins, after.ins, sync=True,
                                             reason="recv after phase")
                return gl

            def bn_affine(gl, rows, g_ap, b_ap, sfx):
                def t1(tag):
                    return const.tile([rows, 1], f32, tag=f"{tag}{sfx}",
                                      name=f"{tag}{sfx}")
                mean, var, y, vh, tmp = (t1(x) for x in
                                         ("mn", "vr", "y", "vh", "tm"))
                nc.vector.tensor_scalar_mul(mean[:], gl[0:rows, 0:1],
                                            1.0 / COUNT)
                nc.vector.tensor_tensor(var[:], mean[:], mean[:], ALU.mult)
                nc.vector.scalar_tensor_tensor(
                    var[:], gl[0:rows, 1:2], 1.0 / COUNT, var[:],
                    ALU.mult, ALU.subtract)
                nc.vector.tensor_scalar_add(var[:], var[:], EPS)
                iv = var[:].bitcast(mybir.dt.int32)
                yi = y[:].bitcast(mybir.dt.int32)
                nc.vector.tensor_scalar(yi, iv, 1, None, ALU.arith_shift_right)
                nc.vector.tensor_scalar(yi, yi, -1, None, ALU.bitwise_xor)
                nc.vector.tensor_scalar(yi, yi, 0x5f3759df + 1, None, ALU.add)
                nc.vector.tensor_scalar_mul(vh[:], var[:], 0.5)
                for _ in range(2):
                    nc.vector.tensor_tensor(tmp[:], y[:], y[:], ALU.mult)
                    nc.vector.tensor_tensor(tmp[:], tmp[:], vh[:], ALU.mult)
                    nc.vector.tensor_scalar(tmp[:], tmp[:], -1.0, 1.5,
                                            ALU.mult, ALU.add)
                    nc.vector.tensor_tensor(y[:], y[:], tmp[:], ALU.mult)
                sc = t1("sc")
                nc.vector.tensor_tensor(sc[:], g_ap, y[:], ALU.mult)
                bi = t1("bi")
                nc.vector.tensor_tensor(bi[:], mean[:], sc[:], ALU.mult)
                nc.vector.tensor_tensor(bi[:], b_ap, bi[:], ALU.subtract)
                return sc, bi

            ex_send(0, "A1", 128)

            # ---- conv1 overflow: M-packed + PE recomb ----
            c1b_last = None
            for n in range(NL):
                sb = sbp.tile([NB1, PLANE], bf16, tag="sb81", name="sb81")
                for k in range(NBCH):
                    c0 = CHUNK * k
                    F = min(CHUNK, PLANE - c0)
                    pt = psB.tile([NB1, CHUNK], f32, tag="psB", name="psB")
                    for j in range(2):
                        nc.tensor.matmul(
                            pt[0:NB1, 0:F], wcol(f"c1o_{j}", 128, NB1),
                            x_pad[j][n][:, c0:c0 + F],
                            start=(j == 0), stop=(j == 1))
                    nc.scalar.activation(sb[:, c0:c0 + F], pt[0:NB1, 0:F],
                                         AF.Copy)
                for k in range(NCHUNK):
                    pt = psR.tile([B1, 8 * W], f32, tag="psR", name="psR")
                    for t in range(9):
                        ty, tx = t // 3, t % 3
                        dq = (ty - 1) * PW + (tx - 1)
                        off = STRIP0 + CHUNK * k + dq
                        rhs = sb[0:NB1, off:off + CHUNK].rearrange(
                            "p (r c) -> p r c", c=PW)[:, :, 0:W]
                        nc.tensor.matmul(pt[:], wcol(f"sel_{t}", NB1, B1),
                                         rhs, start=(t == 0), stop=(t == 8))
                    src_int = pt[:, 0:8 * W].rearrange("p (r c) -> p r c", c=W)
                    dst_int = interior(h1o_img(n), (1 + 8 * k) * PW + 1, 8)
                    col = n * NCHUNK + k
                    nc.scalar.activation(
                        dst_int, src_int, AF.Copy,
                        accum_out=accs[("B1", "s")][:, col:col + 1])
                    sq = sqp.tile([128, 8 * W], f32, tag="sq", name="sq")
                    c1b_last = nc.scalar.activation(
                        sq[0:B1, :].rearrange("p (r c) -> p r c", c=W),
                        dst_int, AF.Square,
                        accum_out=accs[("B1", "q")][:, col:col + 1])

            # ---- masked-out tail: out = relu(x + beta2) (no BN2 dep) ----
            for n in range(NL):
                for rh in range(2):
                    r0 = rh * HALF_ROWS
                    xv = interior(x_pad[1][n], (r0 + 1) * PW + 1, HALF_ROWS)
                    ot = otp.tile([128, HALF_ELEMS], f32, tag="ot", name="ot")
                    otv = ot[:, :].rearrange("p (r c) -> p r c", c=W)
                    mt_last = nc.vector.tensor_scalar(
                        otv, xv, aff_sb[:, 8:9], 0.0, ALU.add, ALU.max)
                    nc.sync.dma_start(
                        out_d[n, 128 + B2:C, r0:r0 + HALF_ROWS, :],
                        ot[B2:128, :])

            sB1 = ex_send(1, "B1", B1, after=mt_last)

            # ---- BN1 main: recv, affine, apply.  The recv/affine (DVE)
            # may run as soon as stats arrive, but the ACT applies are
            # pinned after conv1B's last evac so the scheduler cannot
            # place them ahead of conv1B's ACT stream. ----
            gl = ex_recv(0, 128, sB1)
            s1m, b1m = bn_affine(gl, 128, aff_sb[:, 0:1], aff_sb[:, 1:2], "1m")
            apA_last = None
            for n in range(NL):
                v = interior(h1m[n], STRIP0, H)
                apA_last = nc.scalar.activation(v, v, AF.Relu, bias=b1m[:],
                                                scale=s1m[:])
                bass_mod._add_dep_helper(apA_last.ins, c1b_last.ins,
                                         sync=True,
                                         reason="applyA after conv1B")

            # ---- BN1 ovf: recv, affine (pinned after applyA so the
            # blocked recv cannot split the affineA->applyA chain) ----
            glb = ex_recv(1, B1, apA_last)
            s1o, b1o = bn_affine(glb, B1, aff_sb[0:B1, 2:3],
                                 aff_sb[0:B1, 3:4], "1o")

            # presh: tap-shifted replicas of the (post-BN) overflow planes
            presh = {}

            def replicate(n):
                pr = prp.tile([NB1, PLANE], bf16, tag="pr", name="pr")
                presh[n] = pr
                for t in range(9):
                    ty, tx = t // 3, t % 3
                    dq = (ty - 1) * PW + (tx - 1)
                    src = h1o[0:B1, GUARD + n * PLANE + dq:
                              GUARD + n * PLANE + dq + PLANE]
                    nc.sync.dma_start(pr[9 * t:9 * t + B1, :], src)

            # ---- conv2 ----
            def c2_mms(n, k, nm_main, nm_ovf, co):
                out = []
                for t in range(9):
                    ty, tx = t // 3, t % 3
                    dq = (ty - 1) * PW + (tx - 1)
                    off = STRIP0 + CHUNK * k + dq
                    rhs = h1m[n][:, off:off + CHUNK].rearrange(
                        "p (r c) -> p r c", c=PW)[:, :, 0:W]
                    out.append((wcol(f"{nm_main}_{t}", 128, co), rhs))
                if nm_ovf is not None:
                    off = STRIP0 + CHUNK * k
                    rhs = presh[n][:, off:off + CHUNK].rearrange(
                        "p (r c) -> p r c", c=PW)[:, :, 0:W]
                    out.append((wcol(nm_ovf, NB1, co), rhs))
                return out

            def h2m_dst(n, k):
                return h2m[n][:, 8 * k * W:(8 * k + 8) * W].rearrange(
                    "p (r c) -> p r c", c=W)

            # conv2 main group, main-ci only (no stats yet): the overflow
            # contribution is added afterwards so no part of conv2A waits
            # on the BN1-ovf exchange.
            main_last = None
            for n in range(NL):
                for k in range(NCHUNK):
                    pt = psA.tile([128, 8 * W], f32, tag="ps", name="ps")
                    mms = c2_mms(n, k, "c2m", None, 128)
                    for idx, (lh, rhs) in enumerate(mms):
                        nc.tensor.matmul(pt[:], lh, rhs, start=(idx == 0),
                                         stop=(idx == len(mms) - 1))
                    main_last = nc.scalar.activation(
                        h2m_dst(n, k),
                        pt[:, 0:8 * W].rearrange("p (r c) -> p r c", c=W),
                        AF.Copy)

            # BN1-ovf apply (pinned after the conv2A-main evacs so the
            # scheduler cannot stall them on the exchange) + replication
            for n in range(NL):
                v = interior(h1o_img(n), STRIP0, H)
                ap = nc.scalar.activation(v, v, AF.Relu, bias=b1o[:],
                                          scale=s1o[:])
                bass_mod._add_dep_helper(ap.ins, main_last.ins, sync=True,
                                         reason="applyB after conv2A main")
            # fused per-image pass: overflow-ci contribution + stats for
            # the conv2 main group, then the B2 output group — ONE presh
            # replication per image (was two), and A2 stats finish early
            # enough that the A2 exchange flies before B2's.
            a2_last = None
            c2b_last = None
            for n in range(NL):
                replicate(n)
                for k in range(NCHUNK):
                    pt = psA.tile([128, 8 * W], f32, tag="ps", name="ps")
                    off = STRIP0 + CHUNK * k
                    rhs = presh[n][:, off:off + CHUNK].rearrange(
                        "p (r c) -> p r c", c=PW)[:, :, 0:W]
                    nc.tensor.matmul(pt[:], wcol("c2mo", NB1, 128), rhs,
                                     start=True, stop=True)
                    ob = sqp.tile([128, 8 * W], bf16, tag="sqb", name="sqb")
                    obv = ob[:, :].rearrange("p (r c) -> p r c", c=W)
                    nc.scalar.activation(
                        obv, pt[:, 0:8 * W].rearrange("p (r c) -> p r c",
                                                      c=W), AF.Copy)
                    h2v = h2m_dst(n, k)
                    nc.vector.tensor_tensor(h2v, h2v, obv, ALU.add)
                    col = n * NCHUNK + k
                    sq = sqp.tile([128, 8 * W], f32, tag="sq", name="sq")
                    sqv = sq[:, :].rearrange("p (r c) -> p r c", c=W)
                    nc.scalar.activation(
                        sqv, h2v, AF.Copy,
                        accum_out=accs[("A2", "s")][:, col:col + 1])
                    a2_last = nc.scalar.activation(
                        sqv, h2v, AF.Square,
                        accum_out=accs[("A2", "q")][:, col:col + 1])
                for k in range(NCHUNK):
                    pt = psR.tile([B1, 8 * W], f32, tag="psR", name="psR")
                    mms = c2_mms(n, k, "c2b", "c2bo", B2)
                    for idx, (lh, rhs) in enumerate(mms):
                        nc.tensor.matmul(pt[0:B2, :], lh, rhs,
                                         start=(idx == 0),
                                         stop=(idx == len(mms) - 1))
                    src_int = pt[0:B2, 0:8 * W].rearrange(
                        "p (r c) -> p r c", c=W)
                    col = n * NCHUNK + k
                    dst = h2o[0:B2, n * HW + 8 * k * W:
                              n * HW + (8 * k + 8) * W].rearrange(
                        "p (r c) -> p r c", c=W)
                    nc.scalar.activation(
                        dst, src_int, AF.Copy,
                        accum_out=accs[("B2", "s")][:, col:col + 1])
                    sq = sqp.tile([128, 8 * W], f32, tag="sq", name="sq")
                    c2b_last = nc.scalar.activation(
                        sq[0:B2, :].rearrange("p (r c) -> p r c", c=W),
                        dst, AF.Square,
                        accum_out=accs[("B2", "q")][:, col:col + 1])

            ex_send(2, "A2", 128)

            # B2 stats go out as soon as conv2B finishes; the tail-A chain
            # below then runs during the B2 exchange flight.  recvA2 is
            # pinned after the B2 send so the scheduler cannot float the
            # B2 reduces past the tail-A chain.
            sB2 = ex_send(3, "B2", B2)

            # ---- BN2 main: recv, affine, tail (128 kept, all-DVE) ----
            gl2 = ex_recv(2, 128, sB2)
            s2m, b2m = bn_affine(gl2, 128, aff_sb[:, 4:5], aff_sb[:, 5:6],
                                 "2m")
            tailA_last = None
            for n in range(NL):
                for rh in range(2):
                    r0 = rh * HALF_ROWS
                    xv = interior(x_pad[0][n], (r0 + 1) * PW + 1, HALF_ROWS)
                    h2v = h2m[n][:, r0 * W:r0 * W + HALF_ELEMS].rearrange(
                        "p (r c) -> p r c", c=W)
                    ot = otp.tile([128, HALF_ELEMS], f32, tag="ot", name="ot")
                    otv = ot[:, :].rearrange("p (r c) -> p r c", c=W)
                    nc.vector.scalar_tensor_tensor(
                        otv, h2v, s2m[:], xv, ALU.mult, ALU.add)
                    tailA_last = nc.vector.tensor_scalar(
                        ot[:], ot[:], b2m[:], 0.0, ALU.add, ALU.max)
                    nc.sync.dma_start(
                        out_d[n, 0:128, r0:r0 + HALF_ROWS, :], ot[:])

            # ---- BN2 ovf: recv (pinned after tail-A so the blocked recv
            # cannot split the affineA2->tailA chain), affine, tail ----
            gl2o = ex_recv(3, B2, tailA_last)
            s2o, b2o = bn_affine(gl2o, B2, aff_sb[0:B2, 6:7],
                                 aff_sb[0:B2, 7:8], "2o")
            for n in range(NL):
                for rh in range(2):
                    r0 = rh * HALF_ROWS
                    xv = interior(x_pad[1][n], (r0 + 1) * PW + 1, HALF_ROWS)
                    h2v = h2o[0:B2, n * HW + r0 * W:
                              n * HW + r0 * W + HALF_ELEMS].rearrange(
                        "p (r c) -> p r c", c=W)
                    ot = otp.tile([128, HALF_ELEMS], f32, tag="ot", name="ot")
                    otv = ot[:, :].rearrange("p (r c) -> p r c", c=W)
                    nc.vector.scalar_tensor_tensor(
                        otv[0:B2], h2v, s2o[:], xv[0:B2], ALU.mult, ALU.add)
                    nc.vector.tensor_scalar(ot[0:B2, :], ot[0:B2, :],
                                            b2o[:], 0.0, ALU.add, ALU.max)
                    nc.sync.dma_start(
                        out_d[n, 128:128 + B2, r0:r0 + HALF_ROWS, :],
                        ot[0:B2, :])

    for bi, sem, val in deferred_waits:
        patched = False
        for w in bi.ins.sync_info.on_wait:
            if w.id == sem.num and w.wait_value == 0:
                w.wait_value = val
                patched = True
                break
        assert patched, f"deferred wait not found on {bi.ins.name}"

    nc.compile()
    return nc


def kernel(x, W1, W2, gamma1, beta1, gamma2, beta2, mask1, mask2,
           _trace=False, _trace_kwargs=None):
    from concourse.bass_utils import run_bass_kernel_spmd

    wt, aff, plan, offs = _pack(W1, W2, gamma1, beta1, gamma2, beta2,
                                mask1, mask2)
    k1m, k1o, k2m, k2o, masked2, P = plan

    key = (len(k1o), len(k2o), wt.shape[1])
    if _cache.get("key") != key:
        _cache["nc"] = _build(len(k1o), len(k2o), offs, wt.shape[1])
        _cache["key"] = key
    nc = _cache["nc"]

    x = np.ascontiguousarray(np.asarray(x, np.float32)[:, P])

    in_maps = [{"x": x[i * NL:(i + 1) * NL], "wt": wt, "aff": aff}
               for i in range(N_CORES)]
    kw = {}
    if _trace:
        kw = dict(trace=True, **(_trace_kwargs or {}))
    res = run_bass_kernel_spmd(nc, in_maps, core_ids=list(range(N_CORES)),
                               **kw)
    out_p = np.concatenate([res.results[i]["out"] for i in range(N_CORES)],
                           axis=0)
    _cache["last_results"] = res
    inv = np.empty(C, np.int64)
    inv[P] = np.arange(C)
    return np.ascontiguousarray(out_p[:, inv])
